# revision 24
# baseline (speedup 1.0000x reference)
"""GCN classifier Trainium2 kernel (8-core SPMD, Bass/Tile).

Model (reference):
    h1 = relu(gcnconv(x, W1, b1));  h2 = gcnconv(h1, W2, b2);  out = mean-pool(h2, batch)

Distribution strategy (no cross-core communication):
  * Nodes sharded contiguously across 8 cores (6250 each); x replicated (fp16).
  * Within each core, nodes are greedily re-assigned to 58 dst blocks (128
    lanes each, ~108 nodes used) so that per-(block, src-half) edge loads are
    balanced -> uniform T2=6 edge tiles per bucket (vs 8 for the naive
    contiguous layout).  Selection work, gather bytes and PE scatter matmuls
    all scale with the tile count.
  * Layer-1 aggregation per dst shard. Edge rows are fetched with BULK
    dma_gather (gpsimd library), split into lo/hi source halves because
    gather indices are int16. Per 128-edge tile, the selection matrix
    S[e,d] = w_e * (dstl_e == d) (w_e = dinv[src]*dinv[dst]) is built in one
    fused DVE tensor_scalar; the scatter-add is a single fp16 matmul
    out1T[feat,dst] += X_tile^T @ S accumulating in f32 PSUM.  Producing out1
    TRANSPOSED feeds W1 directly -- no PE transpose round-trip.
  * Per-block tail work is done on PAIRS of blocks (256-wide PSUM tiles) to
    halve Activation-engine instruction count.
  * h1^T and z2 = h1 @ W2 stay on-chip (PSUM/SBUF), never round-trip HBM.
  * Layer 2 + mean-pool collapse algebraically:
        pool_sums[g,f] = sum_e w_e * z2[src_e, f] * [batch[dst_e] == g]
                       = sum_n C[g,n] * z2[n,f]
    with C built host-side from indices/weights only -> dense matmuls, zero
    communication. Host sums 8 partials, divides by counts, adds b2.

Numerics: fp16 operand quantization (~5e-4) with exact f32 PSUM accumulation.
"""

import math
import numpy as np

N_NODES = 50000
N_EDGES = 600000
N_GRAPHS = 64
IN_DIM = 128
HID_DIM = 128
OUT_DIM = 64
N_CORES = 8
P = 128
N_BLOCKS = 58       # dst blocks per core (128 lanes each; 7424 slots >= 6250)
GB = 4              # blocks per gather group
HALF = N_NODES // 2


# ---------------------------------------------------------------- host prep
def _balance_blocks(DST, half_of, n, shard, n_blocks):
    """Greedy per-core node->block assignment balancing (block, half) loads.

    Returns blk_of_node[n], lane_of_node[n]."""
    dl = np.bincount(DST[half_of == 0], minlength=n).astype(np.int64)
    dh = np.bincount(DST[half_of == 1], minlength=n).astype(np.int64)
    blk_of_node = np.zeros(n, dtype=np.int64)
    lane_of_node = np.zeros(n, dtype=np.int64)
    for c in range(N_CORES):
        ids = np.arange(c * shard, (c + 1) * shard)
        order = ids[np.argsort(-(dl[ids] + dh[ids]), kind="stable")]
        lo = np.zeros(n_blocks, dtype=np.int64)
        hi = np.zeros(n_blocks, dtype=np.int64)
        cnt = np.zeros(n_blocks, dtype=np.int64)
        BIG = 1 << 40
        for nid in order:
            a, b = dl[nid], dh[nid]
            score = np.maximum(lo + a, hi + b) * 64 + (lo + a) + (hi + b)
            score[cnt >= P] = BIG
            blk = int(np.argmin(score))
            blk_of_node[nid] = blk
            lane_of_node[nid] = cnt[blk]
            lo[blk] += a
            hi[blk] += b
            cnt[blk] += 1
    return blk_of_node, lane_of_node


def _layout(n_blocks, budgets):
    """Contiguous tile-column layout over gather groups with per-(block,half)
    tile budgets.  budgets: [n_blocks, 2] ints.

    Returns (nb_g, base_gh{(g,h)->col}, offb[n_blocks,2], blocks_before, ntiles)."""
    nb_g = []
    rest = n_blocks
    while rest > 0:
        take = GB if rest > GB + 2 else min(rest, 4)
        nb_g.append(take)
        rest -= take
    base_gh = {}
    offb = np.zeros((n_blocks, 2), dtype=np.int64)
    blocks_before = []
    acc = 0
    bstart = 0
    for g, nb in enumerate(nb_g):
        blocks_before.append(bstart)
        for h in range(2):
            base_gh[(g, h)] = acc
            off = 0
            for b in range(bstart, bstart + nb):
                offb[b, h] = off
                off += int(budgets[b][h])
            acc += off
        bstart += nb
    return nb_g, base_gh, offb, blocks_before, acc


def _host_prep(x, edge_index, batch):
    n = x.shape[0]
    half = n // 2
    shard = n // N_CORES                    # 6250
    n_blocks = N_BLOCKS

    src = np.asarray(edge_index[0], dtype=np.int64)
    dst = np.asarray(edge_index[1], dtype=np.int64)
    batch = np.asarray(batch, dtype=np.int64)

    deg = np.bincount(dst, minlength=n).astype(np.float32) + np.float32(1.0)
    dinv = (np.float32(1.0) / np.sqrt(deg)).astype(np.float32)

    loops = np.arange(n, dtype=np.int64)
    SRC = np.concatenate([src, loops])
    DST = np.concatenate([dst, loops])
    W = (dinv[SRC] * dinv[DST]).astype(np.float32)
    E = SRC.shape[0]

    half_of = SRC // half
    blk_of_node, lane_of_node = _balance_blocks(DST, half_of, n, shard, n_blocks)

    # ---- bucket edges by (core, block, src-half) of DST
    core_of = DST // shard
    blk_of = blk_of_node[DST]
    dstl = lane_of_node[DST]

    n_buckets_per_core = n_blocks * 2
    bucket = (core_of * n_blocks + blk_of) * 2 + half_of
    order = np.argsort(bucket, kind="stable")
    bucket_s = bucket[order]
    counts = np.bincount(bucket_s, minlength=N_CORES * n_buckets_per_core)
    # per-(block, half) tile budget = cross-core max (identical layout on
    # every core; a balancer miss grows the budget, never breaks anything)
    need = np.ceil(counts.reshape(N_CORES, n_blocks, 2) / P).astype(np.int64)
    budgets = need.max(axis=0)              # [n_blocks, 2]

    cum = np.zeros(N_CORES * n_buckets_per_core + 1, dtype=np.int64)
    np.cumsum(counts, out=cum[1:])
    pos = np.arange(E) - cum[bucket_s]

    nb_g, base_gh, offb, blocks_before, ntiles = _layout(n_blocks, budgets)
    g_of_blk = np.concatenate(
        [np.full(nb, g, dtype=np.int64) for g, nb in enumerate(nb_g)])
    base_bh = np.zeros((n_blocks, 2), dtype=np.int64)
    for b in range(n_blocks):
        for h in range(2):
            base_bh[b, h] = base_gh[(g_of_blk[b], h)] + offb[b, h]

    e_core = core_of[order]
    e_blk = blk_of[order]
    e_half = half_of[order]
    col = base_bh[e_blk, e_half] + pos // P
    row = pos % P

    w_cols = np.zeros((N_CORES, P, ntiles), dtype=np.float32)
    dstl_cols = np.zeros((N_CORES, P, ntiles), dtype=np.float32)
    w_cols[e_core, row, col] = W[order]
    dstl_cols[e_core, row, col] = dstl[order].astype(np.float32)

    # gather indices: flat slot i = col*128 + row; idx layout [16, i//16] tiled
    flat_idx = np.zeros((N_CORES, ntiles * P), dtype=np.int16)
    slot = col * P + row
    flat_idx[e_core, slot] = (SRC[order] % half).astype(np.int16)
    nic = ntiles * P // 16                  # int16 idx columns per core
    gidx16 = flat_idx.reshape(N_CORES, nic, 16).transpose(0, 2, 1)  # [C,16,nic]
    gidx = np.ascontiguousarray(
        np.tile(gidx16, (1, 8, 1)))         # replicate to [C, 128, nic]

    # ---- layer-2 dense matrix C[g, n] = sum_{e: src=n} w_e * [batch[dst_e]=g]
    g_of = batch[DST]
    idx = (((SRC // shard) * n_blocks + blk_of_node[SRC]) * P
           + lane_of_node[SRC]) * N_GRAPHS + g_of
    C = np.bincount(idx, weights=W.astype(np.float64),
                    minlength=N_CORES * n_blocks * P * N_GRAPHS)
    C = C.reshape(N_CORES, n_blocks, P, N_GRAPHS)
    CT_cols = np.ascontiguousarray(
        C.transpose(0, 2, 1, 3).reshape(N_CORES, P, n_blocks * N_GRAPHS)
    ).astype(np.float16)

    graph_counts = np.bincount(batch, minlength=N_GRAPHS).astype(np.float32)

    return dict(budgets=tuple(map(tuple, budgets.tolist())),
                n_blocks=n_blocks, shard=shard,
                w_cols=w_cols, dstl_cols=dstl_cols, gidx=gidx,
                CT_cols=CT_cols, graph_counts=graph_counts)


# ---------------------------------------------------------------- bass program
_PROGRAM_CACHE = {}


def _build_program(budgets, n_blocks, n_nodes, repeat=1):
    import concourse.bacc as bacc
    import concourse.tile as tile
    from concourse import mybir

    f32, i32 = mybir.dt.float32, mybir.dt.int32
    f16, i16 = mybir.dt.float16, mybir.dt.int16
    AF = mybir.ActivationFunctionType

    half = n_nodes // 2
    nb_g, base_gh, offb, blocks_before, ntiles = _layout(n_blocks, budgets)
    n_groups = len(nb_g)
    nic = ntiles * P // 16

    nc = bacc.Bacc("TRN2", target_bir_lowering=False, debug=False,
                   num_devices=N_CORES)
    x16_d = nc.dram_tensor("x16", [n_nodes, IN_DIM], f16, kind="ExternalInput")
    w1_d = nc.dram_tensor("w1", [IN_DIM, HID_DIM], f16, kind="ExternalInput")
    w2_d = nc.dram_tensor("w2", [HID_DIM, OUT_DIM], f16, kind="ExternalInput")
    b1_d = nc.dram_tensor("b1", [HID_DIM, 1], f32, kind="ExternalInput")
    iota_d = nc.dram_tensor("iota16", [P, P], f16, kind="ExternalInput")
    gidx_d = nc.dram_tensor("gidx", [P, nic], i16, kind="ExternalInput")
    wc_d = nc.dram_tensor("w_cols", [P, ntiles], f32, kind="ExternalInput")
    dstc_d = nc.dram_tensor("dstl_cols", [P, ntiles], f32, kind="ExternalInput")
    ctc_d = nc.dram_tensor("ct_cols", [P, n_blocks * N_GRAPHS], f16,
                           kind="ExternalInput")
    pool_d = nc.dram_tensor("pool_out", [N_GRAPHS, OUT_DIM], f32,
                            kind="ExternalOutput")

    blocks_before = np.cumsum([0] + nb_g[:-1])

    with tile.TileContext(nc) as tc:
        with (
            tc.tile_pool(name="const", bufs=1) as cp,
            tc.tile_pool(name="work", bufs=8) as wp,
            tc.tile_pool(name="gat", bufs=4) as gp,
            tc.tile_pool(name="ps_out1", bufs=2, space="PSUM") as ps1,
            tc.tile_pool(name="ps_misc", bufs=2, space="PSUM") as ps2,
            tc.tile_pool(name="ps_pool", bufs=1, space="PSUM") as psp,
        ):
            # constants, ordered by first use: group-0 gather indices and
            # the DVE selection inputs first, the pool matrix last.
            nic0 = int(base_gh[(1, 0)]) * 8     # idx cols of group 0
            gidx0 = cp.tile([P, nic0], i16)
            nc.sync.dma_start(out=gidx0[:], in_=gidx_d[:, 0:nic0])
            iota16 = cp.tile([P, P], f16)
            nc.sync.dma_start(out=iota16[:], in_=iota_d[:])
            wc = cp.tile([P, ntiles], f32)
            nc.sync.dma_start(out=wc[:], in_=wc_d[:])
            dstc = cp.tile([P, ntiles], f32)
            nc.sync.dma_start(out=dstc[:], in_=dstc_d[:])
            gidxR = cp.tile([P, nic - nic0], i16)
            nc.sync.dma_start(out=gidxR[:], in_=gidx_d[:, nic0:nic])
            w1_t = cp.tile([IN_DIM, HID_DIM], f16)
            nc.sync.dma_start(out=w1_t[:], in_=w1_d[:])
            w2_t = cp.tile([HID_DIM, OUT_DIM], f16)
            nc.sync.dma_start(out=w2_t[:], in_=w2_d[:])
            b1_t = cp.tile([HID_DIM, 1], f32)
            nc.sync.dma_start(out=b1_t[:], in_=b1_d[:])
            ctc = cp.tile([P, n_blocks * N_GRAPHS], f16)
            nc.sync.dma_start(out=ctc[:], in_=ctc_d[:])

            x_lo = x16_d[0:half, :]
            x_hi = x16_d[half:n_nodes, :]

            CH = 8                           # tiles per dma_gather
            for _rep in range(repeat):
                pool_ps = psp.tile([N_GRAPHS, OUT_DIM], f32, space="PSUM",
                                   tag="pool_ps")
                for g in range(n_groups):
                    bufs = []
                    for h, src_ap in ((0, x_lo), (1, x_hi)):
                        nt_gh = int(sum(budgets[b][h] for b in range(
                            blocks_before[g], blocks_before[g] + nb_g[g])))
                        buf = gp.tile([P, GB * 6, IN_DIM], f16,
                                      tag=f"gat{h}")
                        gsrc = gidx0 if g == 0 else gidxR
                        goff = int(base_gh[(g, h)]) * 8
                        if g > 0:
                            goff -= nic0
                        for s in range(math.ceil(nt_gh / CH)):
                            t0 = s * CH
                            t1 = min(nt_gh, t0 + CH)
                            ni = (t1 - t0) * P
                            nc.gpsimd.dma_gather(
                                buf[:, t0:t1, :], src_ap,
                                gsrc[:, goff + t0 * 8:goff + t1 * 8],
                                ni, ni, IN_DIM)
                        bufs.append(buf)

                    for p0 in range(0, nb_g[g], 2):
                        out1t2 = ps1.tile([IN_DIM, 2 * P], f32, space="PSUM",
                                          tag="out1t2")
                        for bi in range(2):
                            bg = blocks_before[g] + p0 + bi
                            for h in range(2):
                                tb = int(budgets[bg][h])
                                for j in range(tb):
                                    c = int(base_gh[(g, h)] + offb[bg, h] + j)
                                    stw = wp.tile([P, P], f16, tag="stw")
                                    nc.vector.tensor_scalar(
                                        out=stw[:], in0=iota16[:],
                                        scalar1=dstc[:, c:c + 1],
                                        scalar2=wc[:, c:c + 1],
                                        op0=mybir.AluOpType.is_equal,
                                        op1=mybir.AluOpType.mult)
                                    nc.tensor.matmul(
                                        out=out1t2[:, bi * P:(bi + 1) * P],
                                        lhsT=bufs[h][:, int(offb[bg, h]) + j, :],
                                        rhs=stw[:],
                                        start=(h == 0 and j == 0),
                                        stop=(h == 1 and
                                              j == int(budgets[bg][1]) - 1))

                        # h1T = relu(W1^T OUT1^T + b1); z2 = h1 W2; pool += C^T z2
                        o1t2 = wp.tile([IN_DIM, 2 * P], f16, tag="o1t2")
                        nc.scalar.activation(out=o1t2[:], in_=out1t2[:],
                                             func=AF.Copy)
                        h1t2_ps = ps2.tile([HID_DIM, 2 * P], f32, space="PSUM",
                                           tag="h1t2")
                        nc.tensor.matmul(out=h1t2_ps[:], lhsT=w1_t[:],
                                         rhs=o1t2[:], start=True, stop=True)
                        h1t2 = wp.tile([HID_DIM, 2 * P], f16, tag="h1t2_sb")
                        nc.scalar.activation(out=h1t2[:], in_=h1t2_ps[:],
                                             func=AF.Relu, bias=b1_t[:, :1])
                        z2_ps2 = ps2.tile([P, 2 * OUT_DIM], f32, space="PSUM",
                                          tag="z2p")
                        for bi in range(2):
                            nc.tensor.matmul(
                                out=z2_ps2[:, bi * OUT_DIM:(bi + 1) * OUT_DIM],
                                lhsT=h1t2[:, bi * P:(bi + 1) * P],
                                rhs=w2_t[:], start=True, stop=True)
                        z2s2 = wp.tile([P, 2 * OUT_DIM], f16, tag="z2s2")
                        nc.scalar.activation(out=z2s2[:], in_=z2_ps2[:],
                                             func=AF.Copy)
                        for bi in range(2):
                            bg = int(blocks_before[g]) + p0 + bi
                            nc.tensor.matmul(
                                out=pool_ps[:],
                                lhsT=ctc[:, bg * N_GRAPHS:(bg + 1) * N_GRAPHS],
                                rhs=z2s2[:, bi * OUT_DIM:(bi + 1) * OUT_DIM],
                                start=(bg == 0),
                                stop=(bg == n_blocks - 1))

                pool_sb = wp.tile([N_GRAPHS, OUT_DIM], f32, tag="pool_sb")
                nc.scalar.activation(out=pool_sb[:], in_=pool_ps[:],
                                     func=AF.Copy)
                nc.sync.dma_start(out=pool_d[:], in_=pool_sb[:])

    nc.compile()
    return nc


def _make_in_maps(x, W1, W2, b1, prep):
    x16 = np.ascontiguousarray(x.astype(np.float16))
    b1_col = np.ascontiguousarray(b1.reshape(HID_DIM, 1).astype(np.float32))
    w1_16 = W1.astype(np.float16)
    w2_16 = W2.astype(np.float16)
    iota16 = np.tile(np.arange(P, dtype=np.float16)[None, :], (P, 1))
    in_maps = []
    for c in range(N_CORES):
        in_maps.append({
            "x16": x16,
            "w1": w1_16,
            "w2": w2_16,
            "b1": b1_col,
            "iota16": iota16,
            "gidx": np.ascontiguousarray(prep["gidx"][c]),
            "w_cols": np.ascontiguousarray(prep["w_cols"][c]),
            "dstl_cols": np.ascontiguousarray(prep["dstl_cols"][c]),
            "ct_cols": np.ascontiguousarray(prep["CT_cols"][c]),
        })
    return in_maps


# ---------------------------------------------------------------- entry point
def kernel(x, edge_index, batch, W1, b1, W2, b2):
    from concourse.bass_utils import run_bass_kernel_spmd

    x = np.asarray(x, dtype=np.float32)
    W1 = np.asarray(W1, dtype=np.float32)
    b1 = np.asarray(b1, dtype=np.float32)
    W2 = np.asarray(W2, dtype=np.float32)
    b2 = np.asarray(b2, dtype=np.float32)

    prep = _host_prep(x, edge_index, batch)
    key = (prep["budgets"], prep["n_blocks"], x.shape[0])
    if key not in _PROGRAM_CACHE:
        _PROGRAM_CACHE[key] = _build_program(*key)
    nc = _PROGRAM_CACHE[key]

    in_maps = _make_in_maps(x, W1, W2, b1, prep)
    res = run_bass_kernel_spmd(nc, in_maps, list(range(N_CORES)))
    globals()["_LAST_RESULT"] = res

    total = np.zeros((N_GRAPHS, OUT_DIM), dtype=np.float64)
    for c in range(N_CORES):
        total += res.results[c]["pool_out"].astype(np.float64)

    counts = np.maximum(prep["graph_counts"], 1.0).astype(np.float32)
    out = (total.astype(np.float32) / counts[:, None]) + b2[None, :]
    return out.astype(np.float32)
